# revision 4
# baseline (speedup 1.0000x reference)
"""AnchorAttention distributed Bass kernel for 8 TRN2 NeuronCores.

Reference computation (B=2, S=4096, D=1024, H=16, Dh=64, A=512):
  anchors = x[:, :A];  queries = x[:, A:]
  anchor_q/k/v = split_heads(anchors @ Wq/Wk/Wv + b)
  query_q      = split_heads(queries @ Wqt + bqt)
  combined_q   = concat([anchor_q, query_q], axis=2)       # [B,H,S,Dh]
  out  = softmax(combined_q @ anchor_k^T / sqrt(Dh)) @ anchor_v
  out  = merge_heads(out) @ Wo + bo

Sharding: the B*S = 8192 token rows are split into 8 chunks of 1024 rows
(core c -> batch c//4, rows (c%4)*1024). Each core duplicates its
batch's anchor K/V projections, computes Q for its own rows, attention
over the 512 anchors for all 16 heads, and the output projection for its
rows. The output is a pure concatenation: no collectives.

Bias algebra (host-side):
  * bk is dropped entirely: adding bk to K shifts every anchor's score
    for a given row by the same amount, and softmax is shift-invariant.
  * bv is folded into bo: softmax rows sum to 1, so attn @ (V + bv) =
    attn @ V + bv, and (out + bv) @ Wo + bo = out @ Wo + (bv @ Wo + bo).

Layout: everything is kept transposed ([feature, row]) so each matmul
contracts over the partition dim with zero on-chip transposes; the final
output projection naturally lands un-transposed [row, feature] for DMA
out. Host pre-transposes/pre-casts inputs to bf16 (compute dtype; f32
accumulation in PSUM). Softmax row-sums come free via an extra all-ones
column appended to V; no max-subtraction is needed (scores are ~N(0,1),
exp stays in a tiny range; softmax is shift-invariant so results match).

Schedule (the big difference vs the naive phase ordering): input DMAs are
issued in consumption order with xt/wlo split per contraction slice, and
the Q projection runs dt-outer over all 8 PSUM banks so the PE starts as
soon as the first 512KB lands instead of waiting for whole slabs. The
K and V projections are folded INTO the attention pair pipeline (one
head-pair's worth per iteration, one iteration ahead), so their PE work
overlaps the scalar-engine Exp activations, which are the per-pair
bottleneck otherwise.

Heads are packed two per 128-partition tile (head h -> column-tile h//2,
partitions (h%2)*64 ..). Attention is software-pipelined over the 8 head
pairs: scores+exp run one pair ahead of AV, two ahead of normalization.
AV appends an all-ones V column so softmax sums fall out of the matmul;
1/sums (fast DVE reciprocal, lane-parallel at partition bases 0/64) is
partition-broadcast by a PE ones-outer-product written into already-
evacuated rows of the pair's second PSUM tile, and a single mixed-
partition-base DVE multiply writes the normalized attn^T slab.
"""

from contextlib import ExitStack

import numpy as np
import ml_dtypes

import concourse.bass as bass
import concourse.tile as tile
from concourse import bacc, mybir
from concourse import bass_utils

BF16 = mybir.dt.bfloat16
F32 = mybir.dt.float32
B, S, D = 2, 4096, 1024
H, DH = 16, 64
A = 512                  # num_anchor_tokens (asserted at runtime)
RPC = 1024               # rows per core
NCORES = 8
SCALE = 1.0 / np.sqrt(float(DH))

_CACHE = {}


def _build():
    """Build + compile the per-core Bass graph (identical on all cores)."""
    nc = bacc.Bacc("TRN2", target_bir_lowering=False, debug=False)

    xt = nc.dram_tensor("xt", [128, 8, RPC], BF16, kind="ExternalInput")   # rows^T swizzled
    at = nc.dram_tensor("at", [128, 8, A], BF16, kind="ExternalInput")     # anchors^T swizzled
    wlo = nc.dram_tensor("wlo", [128, 8, D], BF16, kind="ExternalInput")   # Q weight rows 0-511
    whi = nc.dram_tensor("whi", [128, 8, D], BF16, kind="ExternalInput")   # Q weight rows 512-1023
    wk = nc.dram_tensor("wk", [128, 8, D], BF16, kind="ExternalInput")
    wv = nc.dram_tensor("wv", [128, 8, D], BF16, kind="ExternalInput")
    wo = nc.dram_tensor("wo", [128, 8, D], BF16, kind="ExternalInput")
    blo = nc.dram_tensor("blo", [128, 8], F32, kind="ExternalInput")
    bhi = nc.dram_tensor("bhi", [128, 8], F32, kind="ExternalInput")
    bo = nc.dram_tensor("bo", [128, D], F32, kind="ExternalInput")   # pre-broadcast bo + bv@Wo

    out = nc.dram_tensor("out", [RPC, D], F32, kind="ExternalOutput")

    Exp = mybir.ActivationFunctionType.Exp

    with tile.TileContext(nc) as tc:
        with tc.tile_pool(name="wpool", bufs=1) as wpool, \
             tc.tile_pool(name="cpool", bufs=1) as cpool, \
             tc.tile_pool(name="kvpool", bufs=1) as kvpool, \
             tc.tile_pool(name="qtpool", bufs=2) as qtpool:
            # x + Q weights live only through the Q projection; their pools
            # (and the 8-bank Q PSUM pool) close before the attention pools
            # open so the attention working set reuses their space.
            projstack = ExitStack()
            wqpool = projstack.enter_context(tc.tile_pool(name="wqpool", bufs=1))
            xpool = projstack.enter_context(tc.tile_pool(name="xpool", bufs=1))
            qpsum = projstack.enter_context(
                tc.tile_pool(name="qpsum", bufs=1, space="PSUM"))

            # ---- DMA issue order == consumption order. xt/wlo are split
            # per dt slice so the dt-outer Q projection starts on slice 0
            # while the rest stream in; everything later is whole-slab. ----
            blo_sb = cpool.tile([128, 8], F32, name="blo_sb")
            nc.sync.dma_start(out=blo_sb, in_=blo.ap())
            bhi_sb = cpool.tile([128, 8], F32, name="bhi_sb")
            nc.sync.dma_start(out=bhi_sb, in_=bhi.ap())

            xt_sb = xpool.tile([128, 8, RPC], BF16, name="xt_sb")
            wlo_sb = wqpool.tile([128, 8, D], BF16, name="wlo_sb")
            whi_sb = wqpool.tile([128, 8, D], BF16, name="whi_sb")
            for k in range(8):
                nc.sync.dma_start(out=xt_sb[:, k:k + 1, :],
                                  in_=xt.ap()[:, k:k + 1, :])
                nc.sync.dma_start(out=wlo_sb[:, k:k + 1, :],
                                  in_=wlo.ap()[:, k:k + 1, :])
            nc.sync.dma_start(out=whi_sb[:, 0:4, :], in_=whi.ap()[:, 0:4, :])
            nc.sync.dma_start(out=whi_sb[:, 4:8, :], in_=whi.ap()[:, 4:8, :])

            wk_sb = wpool.tile([128, 8, D], BF16, name="wk_sb")
            nc.sync.dma_start(out=wk_sb, in_=wk.ap())
            at_sb = wpool.tile([128, 8, A], BF16, name="at_sb")
            nc.sync.dma_start(out=at_sb, in_=at.ap())
            wv_sb = wpool.tile([128, 8, D], BF16, name="wv_sb")
            nc.sync.dma_start(out=wv_sb, in_=wv.ap())
            wo_sb = wpool.tile([128, 8, D], BF16, name="wo_sb")
            nc.sync.dma_start(out=wo_sb, in_=wo.ap())
            bo_bc = cpool.tile([128, D], F32, name="bo_bc")
            nc.sync.dma_start(out=bo_bc, in_=bo.ap())

            ones_bf = cpool.tile([128, DH], BF16, name="ones_bf")
            nc.vector.memset(ones_bf, 1.0)

            # V slab: [128(a%128), ach, head, 65]; cols 0-63 = V head slice,
            # col 64 = ones (supplies softmax row-sums during AV).
            vaug = kvpool.tile([128, 4, H, DH + 1], BF16, name="vaug")
            nc.vector.memset(vaug, 1.0)
            kt_sb = kvpool.tile([128, 8, A], BF16, name="kt_sb")

            qtz = []
            for rc in range(2):
                qt_z0 = qtpool.tile([128, 8, 512], BF16, tag=f"qt0_{rc}",
                                    name=f"qt_z0_{rc}", bufs=1)
                qt_z1 = qtpool.tile([128, 8, 512], BF16, tag=f"qt1_{rc}",
                                    name=f"qt_z1_{rc}", bufs=1)
                nc.vector.memset(qt_z0[64:128, :, :], 0.0)
                nc.vector.memset(qt_z1[0:64, :, :], 0.0)
                qtz.append((qt_z0, qt_z1))

            # ---- Q^T projection per 512-row chunk, dt-OUTER across all 8
            # PSUM banks: matmuls for contraction slice dt only need DMA
            # slice dt of xt/wlo, so compute starts ~2MB earlier. Written
            # into two zero-padded slabs (z0: odd-head partitions zeroed,
            # z1: even) so score matmuls contract over the full 128
            # partitions (FWL stays on, no PE mode switches). ----
            for rc in range(2):
                wsel = wlo_sb if rc == 0 else whi_sb
                bsel = blo_sb if rc == 0 else bhi_sb
                qt_z0, qt_z1 = qtz[rc]
                pq = qpsum.tile([128, 8, 512], F32, tag="pq", name=f"pq{rc}")
                for dt in range(8):
                    for ct in range(8):
                        nc.tensor.matmul(
                            pq[:, ct, :], wsel[:, dt, ct * 128:(ct + 1) * 128],
                            xt_sb[:, dt, rc * 512:(rc + 1) * 512],
                            start=(dt == 0), stop=(dt == 7))
                for ct in range(8):
                    nc.vector.tensor_scalar_add(
                        qt_z0[0:64, ct, :], pq[0:64, ct, :], bsel[0:64, ct:ct + 1])
                    nc.vector.tensor_scalar_add(
                        qt_z1[64:128, ct, :], pq[64:128, ct, :],
                        bsel[64:128, ct:ct + 1])
            qts = qtz
            projstack.close()

            # ---- attention, software-pipelined over the 8 head-pair
            # groups (ct): scores+exp run one group ahead of AV, two ahead
            # of the normalization. The K and V projections for pair ct+1
            # run inside iteration ct (PE work that overlaps the scalar
            # Exp). Both heads of a group share one praw2 slab, one
            # reciprocal, and one [128, 1024] normalize multiply. The
            # 1/sums broadcast is a PE ones-outer-product written into
            # partitions 0-127 of the group's SECOND pav tile (its rows
            # were already evacuated), so no PSUM banks are added and the
            # DVE multiply reads it with mixed partition bases. ----
            attnstack = ExitStack()
            psum = attnstack.enter_context(
                tc.tile_pool(name="psum", bufs=2, space="PSUM"))
            attnpool = attnstack.enter_context(tc.tile_pool(name="attnpool", bufs=1))
            ptpool = attnstack.enter_context(tc.tile_pool(name="ptpool", bufs=8))
            tmppool = attnstack.enter_context(tc.tile_pool(name="tmppool", bufs=4))
            rcppool = attnstack.enter_context(tc.tile_pool(name="rcppool", bufs=3))
            outpool = attnstack.enter_context(tc.tile_pool(name="outpool", bufs=3))
            attnT = attnpool.tile([128, 8, RPC], BF16, name="attnT")

            def kvproj(j):
                # K^T and V projections for head pair j, one PSUM "work"
                # alloc (bank 0 = K^T pair slab, bank 1 = V [a,4x128]).
                kv = psum.tile([128, 2, 512], F32, tag="work", name="kv",
                               bufs=2)
                for dt in range(8):
                    nc.tensor.matmul(
                        kv[:, 0, :], wk_sb[:, dt, j * 128:(j + 1) * 128],
                        at_sb[:, dt, :], start=(dt == 0), stop=(dt == 7))
                kvv = kv[:, 1, :].rearrange("p (a c) -> p a c", a=4)
                for ach in range(4):
                    for dt in range(8):
                        nc.tensor.matmul(
                            kvv[:, ach, :],
                            at_sb[:, dt, ach * 128:(ach + 1) * 128],
                            wv_sb[:, dt, j * 128:(j + 1) * 128],
                            start=(dt == 0), stop=(dt == 7))
                nc.vector.tensor_copy(kt_sb[:, j, :], kv[:, 0, :])
                vsrc = kv[:, 1, :].rearrange("p (a h d) -> p a h d", a=4, h=2)
                nc.vector.tensor_copy(vaug[:, :, 2 * j:2 * j + 2, 0:DH], vsrc)

            def stage_scores(ct):
                st = {"pts": []}
                for par in range(2):
                    for rc in range(2):
                        qt_sb = qts[rc][par]
                        pt = ptpool.tile([128, 4, 512], BF16, tag="pt",
                                         name="pt")
                        for half in range(2):
                            s2 = psum.tile([128, 2, 512], F32, tag="s",
                                           name="s2", bufs=2)
                            for k in range(2):
                                ach = 2 * half + k
                                nc.tensor.matmul(
                                    s2[:, k, :],
                                    kt_sb[:, ct, ach * 128:(ach + 1) * 128],
                                    qt_sb[:, ct, :],
                                    start=True, stop=True)
                            nc.scalar.activation(
                                out=pt[:, 2 * half:2 * half + 2, :], in_=s2,
                                func=Exp, scale=SCALE)
                        st["pts"].append(pt)
                return st

            def stage_av(ct, par, st):
                h = 2 * ct + par
                pav = psum.tile([128, 2, 512], F32, tag="work", name="pav",
                                bufs=2)
                for rc in range(2):
                    pt = st["pts"][par * 2 + rc]
                    for ach in range(4):
                        nc.tensor.matmul(
                            pav[0:DH + 1, rc, :], vaug[:, ach, h, :],
                            pt[:, ach, :], start=(ach == 0), stop=(ach == 3))
                if par == 0:
                    st["praw2"] = tmppool.tile([128, 2, 512], BF16,
                                               tag="praw", name="praw2")
                    # sums gathered to partition bases {0,64} of one tile
                    # so the reciprocal+cast run lane-parallel
                    st["sums4"] = rcppool.tile([128, 2, 512], F32,
                                               tag="sums", name="sums4")
                nc.vector.tensor_copy(st["praw2"][par * 64:par * 64 + DH, :, :],
                                      pav[0:DH, :, :])
                row = par * 64
                nc.vector.tensor_copy(st["sums4"][row:row + 1, :, :],
                                      pav[DH:DH + 1, :, :])
                st[f"pav{par}"] = pav

            def stage_recip(ct, st):
                rcp4 = rcppool.tile([128, 2, 512], F32, tag="rcp",
                                    name="rcp4")
                nc.vector.reciprocal_approx_fast(rcp4, st["sums4"])
                rcpbf = rcppool.tile([128, 2, 512], BF16, tag="rcpbf",
                                     name="rcpbf")
                nc.vector.tensor_copy(rcpbf, rcp4)
                st["rcpbf"] = rcpbf

            def stage_norm(ct, st):
                pav1 = st["pav1"]
                for par in range(2):
                    row = par * 64
                    for rcn in range(2):
                        nc.tensor.matmul(
                            pav1[par * 64:(par + 1) * 64, rcn, :],
                            ones_bf[row:row + 1, :],
                            st["rcpbf"][row:row + 1, rcn, :],
                            start=True, stop=True)
                dst = attnT[:, ct, :].rearrange("p (b r) -> p b r", b=2)
                nc.vector.tensor_mul(dst, st["praw2"], pav1)

            # O-proj partials for the first two tiles are emitted inside
            # the pipeline drain so the PE has work while the last group's
            # normalization chain runs.
            pouts_head = []

            def oproj_partial():
                for nh in range(2):
                    pout = psum.tile([128, 512], F32, tag="work",
                                     name="pout")
                    for ct2 in range(7):
                        nc.tensor.matmul(
                            pout, attnT[:, ct2, 0:128],
                            wo_sb[:, ct2, nh * 512:(nh + 1) * 512],
                            start=(ct2 == 0), stop=False)
                    pouts_head.append(pout)

            kvproj(0)
            sts = {}
            for i in range(10):
                if i < 8:
                    sts[i] = stage_scores(i)
                if i + 1 < 8:
                    kvproj(i + 1)
                if i == 9:
                    oproj_partial()
                if 2 <= i <= 9:
                    stage_recip(i - 2, sts[i - 2])
                    stage_norm(i - 2, sts[i - 2])
                if 1 <= i <= 8:
                    stage_av(i - 1, 0, sts[i - 1])
                    stage_av(i - 1, 1, sts[i - 1])

            # ---- output projection ----
            for rti in range(8):
                for nh in range(2):
                    if rti == 0:
                        pout = pouts_head[nh]
                        nc.tensor.matmul(
                            pout, attnT[:, 7, 0:128],
                            wo_sb[:, 7, nh * 512:(nh + 1) * 512],
                            start=False, stop=True)
                    else:
                        pout = psum.tile([128, 512], F32, tag="work",
                                         name="pout")
                        for ct2 in range(8):
                            nc.tensor.matmul(
                                pout, attnT[:, ct2, rti * 128:(rti + 1) * 128],
                                wo_sb[:, ct2, nh * 512:(nh + 1) * 512],
                                start=(ct2 == 0), stop=(ct2 == 7))
                    out_t = outpool.tile([128, 512], F32, tag="out",
                                         name="out_t")
                    nc.vector.tensor_add(out_t, pout,
                                         bo_bc[:, nh * 512:(nh + 1) * 512])
                    nc.sync.dma_start(
                        out=out.ap()[rti * 128:(rti + 1) * 128,
                                     nh * 512:(nh + 1) * 512],
                        in_=out_t)
            attnstack.close()

    nc.compile()
    return nc


def _swz(a):
    """[1024, cols] -> [128, 8, cols] with row r -> (r % 128, r // 128)."""
    return np.ascontiguousarray(
        a.reshape(8, 128, -1).transpose(1, 0, 2))


def _make_in_maps(x, Wq, bq, Wk, bk, Wv, bv, Wqt, bqt, Wo, bo):
    x = np.asarray(x, dtype=np.float32)
    bf = ml_dtypes.bfloat16

    wq_b = np.ascontiguousarray(np.asarray(Wq, np.float32).astype(bf))
    wqt_b = np.ascontiguousarray(np.asarray(Wqt, np.float32).astype(bf))
    wk_b = np.ascontiguousarray(np.asarray(Wk, np.float32).astype(bf))
    wv_b = np.ascontiguousarray(np.asarray(Wv, np.float32).astype(bf))
    wo_b = np.ascontiguousarray(np.asarray(Wo, np.float32).astype(bf))
    colmaj = lambda v: np.ascontiguousarray(
        np.asarray(v, np.float32).reshape(8, 128).T)
    bq, bqt = map(colmaj, (bq, bqt))
    # bv folded through Wo into the output bias (softmax rows sum to 1);
    # bk dropped (constant score shift per row, softmax-invariant).
    bo_eff = (np.asarray(bo, np.float32)
              + np.asarray(bv, np.float32) @ np.asarray(Wo, np.float32))
    bo_eff = np.ascontiguousarray(np.broadcast_to(bo_eff, (128, D)))

    wq_sw, wqt_sw = _swz(wq_b), _swz(wqt_b)
    wk_sw, wv_sw, wo_sw = _swz(wk_b), _swz(wv_b), _swz(wo_b)
    at_sw = [_swz(x[b, :A, :].T.astype(bf)) for b in range(B)]
    in_maps = []
    for c in range(NCORES):
        b, q = divmod(c, 4)
        rows = x[b, q * RPC:(q + 1) * RPC, :]
        in_maps.append({
            "xt": _swz(rows.T.astype(bf)),
            "at": at_sw[b],
            "wlo": wq_sw if q == 0 else wqt_sw,
            "whi": wqt_sw,
            "wk": wk_sw, "wv": wv_sw, "wo": wo_sw,
            "blo": bq if q == 0 else bqt, "bhi": bqt,
            "bo": bo_eff,
        })
    return in_maps


def kernel(x, Wq, bq, Wk, bk, Wv, bv, Wqt, bqt, Wo, bo, num_anchor_tokens):
    assert int(num_anchor_tokens) == A
    if "nc" not in _CACHE:
        _CACHE["nc"] = _build()
    nc = _CACHE["nc"]

    in_maps = _make_in_maps(x, Wq, bq, Wk, bk, Wv, bv, Wqt, bqt, Wo, bo)
    res = bass_utils.run_bass_kernel_spmd(
        nc, in_maps, core_ids=list(range(NCORES)))
    out = np.empty((B, S, D), np.float32)
    for c in range(NCORES):
        b, q = divmod(c, 4)
        out[b, q * RPC:(q + 1) * RPC, :] = res.results[c]["out"]
    return out
